# revision 13
# baseline (speedup 1.0000x reference)
"""Trainium2 Bass kernel for a GNN message-passing decoder layer.

Reference computation (N=4096 nodes, K=48 neighbors, H=128, E_IN=384):
  h_EV = concat([broadcast(h_V), h_E], -1)          # [N, K, 512]
  h = gelu(h_EV @ W1 + b1); h = gelu(h @ W2 + b2)   # per-edge MLP
  msg = h @ W3 + b3
  dh = sum_k(mask_attend * msg) / 30
  x1 = LN1(h_V + dh)
  dh2 = gelu(x1 @ Win + bin) @ Wout + bout
  out = mask_V * LN2(x1 + dh2)

Sharding: node dimension split across 8 NeuronCores (512 nodes/core), weights
replicated.  Per core, nodes are processed in 4 blocks of 128; edge tokens are
laid out k-major (token = k*128 + n).

Fast path (mask_attend all-ones, which the staged inputs satisfy): the mask
multiply, its [1,TPB] mask-row DMA and the PE rank-1 mask broadcast are all
dropped; the b3 * (sum_k mask) term keeps its general matmul form against a
memset ones tile.  The general masked build is kept as a fallback and
selected at runtime in kernel().

v2 layout/engine choices (driven by the CoreSim cost model):
  - h_E and h_V stream + W1 in fp8e4 (inputs /16, W1 *16 so 0.02-scale
    weights stay out of the fp8 subnormal range); W1 matmuls run in DoubleRow
    perf mode (2 contraction rows per partition, 0.5 PE cycles/col).
  - gelu chunks are [128,1024] (two PSUM banks) to halve Activation-engine
    instruction overhead; gelu outputs bf16.
  - the k-reduce and its accumulation run on DVE in bf16 (packed 2-byte DVE
    perf mode); W2/W3/FFN weights bf16.
  - LayerNorm sqrt is batched: one [128,4] Sqrt per LN stage, so the
    activation table only thrashes Gelu->Sqrt 4x per layer instead of 10x.
  - DMA is issued from two queues (SP for the DoubleRow stream + stores,
    Pool/SWDGE for the plain fp8 stream + small loads) so DGE/seq overheads
    overlap; transfer time itself is a shared-device bottleneck (~360GB/s).
"""

import os
import sys

sys.path.insert(0, "/opt/trn_rl_repo")

import numpy as np

N, K, H, E_IN = 4096, 48, 128, 384
NCORES = 8
NPC = N // NCORES          # nodes per core = 512
NBLK = NPC // 128          # node blocks per core = 4
TPB = K * 128              # tokens per block = 6144
HTPB = TPB // 2            # tokens per half = 3072
SCALE = 30.0
EPS = 1e-5
FP8_S = 16.0               # h/16, W1*16

_CACHE = {}


def _build_nc2(reps=1, triv1=False, triv2=False, trivfb=False):
    """Fast no-mask build: fp8 DoubleRow edge MLP, batched LN.

    triv1/triv2: LN1/LN2 affine is identity (scale 1, offset 0; triv2 also
    needs mask_V all-ones) -> skip the affine (and final mask) ops.
    trivfb: FFN Win bias is zero -> pair the FFN gelus without bias.
    """
    import concourse.bass as bass
    import concourse.mybir as mybir
    from concourse import bacc
    from concourse.bass import ts
    from concourse.tile import TileContext
    from contextlib import ExitStack

    F32 = mybir.dt.float32
    BF16 = mybir.dt.bfloat16
    FP8 = mybir.dt.float8e4
    GELU = mybir.ActivationFunctionType.Gelu
    SQRT = mybir.ActivationFunctionType.Sqrt
    ADD = mybir.AluOpType.add
    AX = mybir.AxisListType.X
    DR = mybir.MatmulPerfMode.DoubleRow

    nc = bacc.Bacc()

    xe = nc.dram_tensor("xe", [NBLK, E_IN, TPB], FP8, kind="ExternalInput")
    hvdr = nc.dram_tensor("hvdr", [64, 2, NPC], FP8, kind="ExternalInput")
    hvt = nc.dram_tensor("hvt", [NBLK, 128, H], F32, kind="ExternalInput")
    w1adr = nc.dram_tensor("w1adr", [64, 2, H], FP8, kind="ExternalInput")
    w1b = nc.dram_tensor("w1b", [128, E_IN], FP8, kind="ExternalInput")
    w2 = nc.dram_tensor("w2", [H, H], BF16, kind="ExternalInput")
    w3 = nc.dram_tensor("w3", [H, H], BF16, kind="ExternalInput")
    win = nc.dram_tensor("win", [H, 4 * H], BF16, kind="ExternalInput")
    wout = nc.dram_tensor("wout", [128, 4 * H], BF16, kind="ExternalInput")
    b1 = nc.dram_tensor("b1", [H, 1], F32, kind="ExternalInput")
    b2 = nc.dram_tensor("b2", [H, 1], F32, kind="ExternalInput")
    b3x48 = nc.dram_tensor("b3x48", [K, H], BF16, kind="ExternalInput")
    binc = nc.dram_tensor("binc", [128, 4], F32, kind="ExternalInput")
    bincr = nc.dram_tensor("bincr", [4, 128], BF16, kind="ExternalInput")
    bout = nc.dram_tensor("bout", [H, 1], F32, kind="ExternalInput")
    s1b = nc.dram_tensor("s1b", [128, H], F32, kind="ExternalInput")
    o1b = nc.dram_tensor("o1b", [128, H], F32, kind="ExternalInput")
    s2mv = nc.dram_tensor("s2mv", [NBLK, 128, H], F32, kind="ExternalInput")
    o2mv = nc.dram_tensor("o2mv", [NBLK, 128, H], F32, kind="ExternalInput")
    ident = nc.dram_tensor("ident", [128, 128], F32, kind="ExternalInput")
    out = nc.dram_tensor("out", [NPC, H], F32, kind="ExternalOutput")

    with TileContext(nc) as tc, ExitStack() as ctx:
        const = ctx.enter_context(tc.tile_pool(name="const", bufs=1))
        xe01p = ctx.enter_context(tc.tile_pool(name="xe01", bufs=2))
        xe2p = ctx.enter_context(tc.tile_pool(name="xe2", bufs=2))
        g1p = ctx.enter_context(tc.tile_pool(name="g1", bufs=2))
        g2p = ctx.enter_context(tc.tile_pool(name="g2", bufs=2))
        rpp = ctx.enter_context(tc.tile_pool(name="rp", bufs=2))
        rbp = ctx.enter_context(tc.tile_pool(name="rb", bufs=2))
        hvrp = ctx.enter_context(tc.tile_pool(name="hvr", bufs=2))
        smp = ctx.enter_context(tc.tile_pool(name="sm", bufs=3))
        x1prp = ctx.enter_context(tc.tile_pool(name="x1pr", bufs=NBLK))
        x1p = ctx.enter_context(tc.tile_pool(name="x1", bufs=NBLK))
        outp = ctx.enter_context(tc.tile_pool(name="outp", bufs=2))
        x2p = ctx.enter_context(tc.tile_pool(name="x2", bufs=NBLK))
        # PSUM: psA 2x[H,1024] (4 banks), psB 1x[H,1024] (2), psT 1 bank,
        # psD 1 bank = 8 banks.  start=True zeroes a whole 2KB bank region,
        # so psT/psD own full banks.
        psA = ctx.enter_context(tc.tile_pool(name="psA", bufs=2, space="PSUM"))
        psB = ctx.enter_context(tc.tile_pool(name="psB", bufs=1, space="PSUM"))
        psT = ctx.enter_context(tc.tile_pool(name="psT", bufs=2, space="PSUM"))
        psD = psT

        def cload(name, dram, shape, dt, eng=None):
            t = const.tile(shape, dt, tag=name)
            (eng or nc.sync).dma_start(out=t, in_=dram[:])
            return t

        # critical-path constants first (edge-MLP weights), then prefetch the
        # first edge tiles so their transfer leads the DMA-device queue, then
        # the tail-phase constants.
        w1adr_t = cload("w1adr", w1adr, [64, 2, H], FP8)
        w1b_t = cload("w1b", w1b, [128, E_IN], FP8)
        b1_t = cload("b1", b1, [H, 1], F32)
        hvdr_t = cload("hvdr", hvdr, [64, 2, NPC], FP8, eng=nc.gpsimd)
        w1b3 = w1b_t[:].rearrange("p (c h) -> p c h", c=3)

        def dma_xe(b, half, split=False):
            xe01_t = xe01p.tile([128, 2, HTPB], FP8, tag="xe01")
            xe2_t = xe2p.tile([128, HTPB], FP8, tag="xe2")
            cuts = [0, 1024, HTPB] if split else [0, HTPB]
            for lo, hi in zip(cuts[:-1], cuts[1:]):
                nc.sync.dma_start(
                    out=xe01_t[:, :, lo:hi],
                    in_=xe[
                        b, 0:256, half * HTPB + lo : half * HTPB + hi
                    ].rearrange("(two p) t -> p two t", two=2),
                )
                nc.gpsimd.dma_start(
                    out=xe2_t[:, lo:hi],
                    in_=xe[b, 256:384, half * HTPB + lo : half * HTPB + hi],
                )
            return xe01_t, xe2_t

        prefetch = {(0, 0): dma_xe(0, 0, split=True), (0, 1): dma_xe(0, 1)}

        w2_t = cload("w2", w2, [H, H], BF16)
        b2_t = cload("b2", b2, [H, 1], F32)
        w3_t = cload("w3", w3, [H, H], BF16)
        b3x48_t = cload("b3x48", b3x48, [K, H], BF16)
        win_t = cload("win", win, [H, 4 * H], BF16)
        wout_t = cload("wout", wout, [128, 4 * H], BF16)
        binc_t = cload("binc", binc, [128, 4], F32)
        bincr_t = cload("bincr", bincr, [4, 128], BF16)
        bout_t = cload("bout", bout, [H, 1], F32)
        s1b_t = cload("s1b", s1b, [128, H], F32)
        o1b_t = cload("o1b", o1b, [128, H], F32)
        ident_t = cload("ident", ident, [128, 128], F32)

        ones48 = const.tile([K, 128], BF16, tag="ones48")
        nc.vector.memset(ones48, 1.0)
        x1F = const.tile([H, NPC], BF16, tag="x1F")
        z_sb = const.tile([128, NBLK, 512], BF16, tag="z_sb")
        mv2a = const.tile([128, NBLK, 2], F32, tag="mv2a")
        mv2b = const.tile([128, NBLK, 2], F32, tag="mv2b")
        eps_t = const.tile([128, 1], F32, tag="eps")
        nc.vector.memset(eps_t, EPS)

        for _rep in range(reps):
            x1pre_tiles = []
            x1_tiles = []
            x2_tiles = []
            for b in range(NBLK):
                hvt_b = smp.tile([128, H], F32, tag="hvt")
                nc.sync.dma_start(out=hvt_b, in_=hvt[b])
                hvrep = hvrp.tile([64, 2, 512], FP8, tag="hvrep")
                for i in range(4):
                    nc.gpsimd.tensor_copy(
                        out=hvrep[:, :, ts(i, 128)], in_=hvdr_t[:, :, ts(b, 128)]
                    )
                rblk = rbp.tile([H, 128], BF16, tag="rblk")

                for half in range(2):
                    xe01_t, xe2_t = prefetch.pop((b, half), None) or dma_xe(b, half)
                    for jj in range(3):
                        ps1 = psA.tile([H, 1024], F32, tag="ps1")
                        for g in range(2):
                            off = jj * 1024 + g * 512
                            o2 = g * 512
                            nc.tensor.matmul(
                                ps1[:, o2 : o2 + 512],
                                w1adr_t[:],
                                hvrep[:],
                                start=True,
                                stop=False,
                                perf_mode=DR,
                            )
                            nc.tensor.matmul(
                                ps1[:, o2 : o2 + 512],
                                w1b3[:, 0:2, :],
                                xe01_t[:, :, off : off + 512],
                                start=False,
                                stop=False,
                                perf_mode=DR,
                            )
                            nc.tensor.matmul(
                                ps1[:, o2 : o2 + 512],
                                w1b_t[:, ts(2, 128)],
                                xe2_t[:, off : off + 512],
                                start=False,
                                stop=True,
                            )
                        g1 = g1p.tile([H, 1024], BF16, tag="g1")
                        nc.scalar.activation(out=g1[:], in_=ps1[:], func=GELU, bias=b1_t[:])
                        ps2 = psB.tile([H, 1024], F32, tag="ps2")
                        nc.tensor.matmul(ps2[:, 0:512], w2_t[:], g1[:, 0:512], start=True, stop=True)
                        nc.tensor.matmul(ps2[:, 512:1024], w2_t[:], g1[:, 512:1024], start=True, stop=True)
                        # gelu writes k-innermost so the k-reduce runs in the
                        # packed-bf16 DVE fast mode
                        g2t = g2p.tile([H, 128, 8], BF16, tag="g2")
                        nc.scalar.activation(
                            out=g2t[:].rearrange("p n k -> p k n"),
                            in_=ps2[:],
                            func=GELU,
                            bias=b2_t[:],
                        )
                        with nc.allow_low_precision(reason="bf16 k-sum, LN downstream"):
                            if half == 0 and jj == 0:
                                nc.vector.tensor_reduce(out=rblk[:], in_=g2t[:], axis=AX, op=ADD)
                            else:
                                rp = rpp.tile([H, 128], BF16, tag="rp")
                                nc.vector.tensor_reduce(out=rp[:], in_=g2t[:], axis=AX, op=ADD)
                                nc.vector.tensor_add(out=rblk[:], in0=rblk[:], in1=rp[:])

                # dh_pre = W3^T r + 48 * b3   (feature-major [h, n])
                psd = psD.tile([H, 512], F32, tag="psT")
                nc.tensor.matmul(psd[:, 0:128], w3_t[:], rblk[:], start=True, stop=False)
                nc.tensor.matmul(psd[:, 0:128], b3x48_t[:], ones48[:], start=False, stop=True)
                dh_sb = smp.tile([H, 128], F32, tag="dh_sb")
                nc.vector.tensor_copy(out=dh_sb[:], in_=psd[:, 0:128])
                psdT = psT.tile([128, 512], F32, tag="psT")
                nc.tensor.transpose(psdT[:, 0:128], dh_sb[:], ident_t[:])
                x1pre = x1prp.tile([128, H], F32, tag="x1pre")
                nc.vector.tensor_add(out=x1pre[:], in0=hvt_b[:], in1=psdT[:, 0:128])
                x1pre_tiles.append(x1pre)
                st6 = smp.tile([128, 6], F32, tag="st6")
                nc.vector.bn_stats(out=st6[:], in_=x1pre[:])
                nc.vector.bn_aggr(out=mv2a[:, b, :], in_=st6[:])

            # batched LN1: one Sqrt for all 4 blocks (a per-block sqrt would
            # interleave with later blocks' gelus and thrash the act table)
            sd4 = const.tile([128, NBLK], F32, tag="sd4")
            nc.scalar.activation(
                out=sd4[:], in_=mv2a[:, :, 1:2], func=SQRT, bias=eps_t[:]
            )
            rstd4 = const.tile([128, NBLK], F32, tag="rstd4")
            nc.vector.reciprocal(out=rstd4[:], in_=sd4[:])
            for b in range(NBLK):
                xn = (x1p if triv1 else smp).tile([128, H], F32, tag="xn")
                nc.vector.tensor_scalar(
                    out=xn[:],
                    in0=x1pre_tiles[b][:],
                    scalar1=mv2a[:, b, 0:1],
                    scalar2=rstd4[:, b : b + 1],
                    op0=mybir.AluOpType.subtract,
                    op1=mybir.AluOpType.mult,
                )
                if triv1:
                    x1 = xn
                else:
                    x1 = x1p.tile([128, H], F32, tag="x1")
                    nc.vector.tensor_mul(out=x1[:], in0=xn[:], in1=s1b_t[:])
                    nc.vector.tensor_add(out=x1[:], in0=x1[:], in1=o1b_t[:])
                x1_tiles.append(x1)
                psxT = psT.tile([H, 512], F32, tag="psT")
                nc.tensor.transpose(psxT[:, 0:128], x1[:], ident_t[:])
                nc.vector.tensor_copy(out=x1F[:, ts(b, 128)], in_=psxT[:, 0:128])

            # per-block FFN: starts as soon as that block's x1F column is
            # written; z gelu has no bias on the trivfb path (bias folded in
            # the general path via a rank-1 matmul against ones)
            ones_row = ones48[0:1, :]
            for b in range(NBLK):
                psz = psA.tile([H, 1024], F32, tag="ps1")
                for c in range(4):
                    nc.tensor.matmul(
                        psz[:, ts(c, 128)],
                        win_t[:, ts(c, 128)],
                        x1F[:, ts(b, 128)],
                        start=True,
                        stop=trivfb,
                    )
                    if not trivfb:
                        nc.tensor.matmul(
                            psz[:, ts(c, 128)],
                            bincr_t[c : c + 1, :],
                            ones_row,
                            start=False,
                            stop=True,
                        )
                zb = z_sb[:, b, :]
                nc.scalar.activation(out=zb, in_=psz[:, 0:512], func=GELU)
                psd2 = psB.tile([H, 1024], F32, tag="ps2")
                for c in range(4):
                    nc.tensor.matmul(
                        psd2[:, 0:128],
                        wout_t[:, ts(c, 128)],
                        z_sb[:, b, ts(c, 128)],
                        start=(c == 0),
                        stop=(c == 3),
                    )
                if triv2:
                    s2mv_b = o2mv_b = None
                else:
                    s2mv_b = smp.tile([128, H], F32, tag="s2mv")
                    nc.gpsimd.dma_start(out=s2mv_b, in_=s2mv[b])
                    o2mv_b = smp.tile([128, H], F32, tag="o2mv")
                    nc.gpsimd.dma_start(out=o2mv_b, in_=o2mv[b])
                dh2 = smp.tile([H, 128], F32, tag="dh2")
                nc.vector.tensor_scalar_add(
                    out=dh2[:], in0=psd2[:, 0:128], scalar1=bout_t[:]
                )
                psd2T = psT.tile([128, 512], F32, tag="psT")
                nc.tensor.transpose(psd2T[:, 0:128], dh2[:], ident_t[:])
                x2 = x2p.tile([128, H], F32, tag="x2")
                nc.vector.tensor_add(out=x2[:], in0=x1_tiles[b][:], in1=psd2T[:, 0:128])
                x2_tiles.append((x2, s2mv_b, o2mv_b))
                st6b = smp.tile([128, 6], F32, tag="st6")
                nc.vector.bn_stats(out=st6b[:], in_=x2[:])
                nc.vector.bn_aggr(out=mv2b[:, b, :], in_=st6b[:])
            sd4b = const.tile([128, NBLK], F32, tag="sd4b")
            rstd4b = const.tile([128, NBLK], F32, tag="rstd4b")
            for b in range(NBLK):
                x2, s2mv_b, o2mv_b = x2_tiles[b]
                nc.scalar.activation(
                    out=sd4b[:, b : b + 1], in_=mv2b[:, b, 1:2], func=SQRT, bias=eps_t[:]
                )
                nc.vector.reciprocal(out=rstd4b[:, b : b + 1], in_=sd4b[:, b : b + 1])
                y = outp.tile([128, H], F32, tag="y")
                nc.vector.tensor_scalar(
                    out=y[:],
                    in0=x2[:],
                    scalar1=mv2b[:, b, 0:1],
                    scalar2=rstd4b[:, b : b + 1],
                    op0=mybir.AluOpType.subtract,
                    op1=mybir.AluOpType.mult,
                )
                if not triv2:
                    nc.vector.tensor_mul(out=y[:], in0=y[:], in1=s2mv_b[:])
                    nc.vector.tensor_add(out=y[:], in0=y[:], in1=o2mv_b[:])
                nc.sync.dma_start(out=out[ts(b, 128), :], in_=y[:])

    nc.finalize()
    return nc


def _prep_inputs2(h_V, h_E, mask_V, W1_w, W1_b, W2_w, W2_b, W3_w, W3_b,
                  Win_w, Win_b, Wout_w, Wout_b, norm1_s, norm1_o, norm2_s, norm2_o):
    import ml_dtypes
    f = np.float32
    BF = ml_dtypes.bfloat16
    F8 = ml_dtypes.float8_e4m3
    h_V = np.asarray(h_V, f)
    h_E = np.asarray(h_E, f)
    mask_V = np.asarray(mask_V, f)
    W1_w = np.asarray(W1_w, f)

    # per-core, k-major edge features (scaled 1/16, fp8): xe[c][b, f, k*128+n]
    xe = np.ascontiguousarray(
        (h_E / FP8_S).reshape(NCORES, NBLK, 128, K, E_IN).transpose(0, 1, 4, 3, 2)
    ).reshape(NCORES, NBLK, E_IN, TPB).astype(F8)
    # h_V DoubleRow stream: hvdr[c][k, i, n] = h_V[n, i*64+k] / 16
    hvdr = np.ascontiguousarray(
        (h_V / FP8_S).reshape(NCORES, NPC, 2, 64).transpose(0, 3, 2, 1)
    ).astype(F8)
    hvt = h_V.reshape(NCORES, NBLK, 128, H)

    # W1 node part DoubleRow: w1adr[k, i, m] = W1[i*64+k, m] * 16
    w1adr = np.ascontiguousarray(
        (W1_w[:H] * FP8_S).reshape(2, 64, H).transpose(1, 0, 2)
    ).astype(F8)
    # W1 edge part: w1b[p, c*H+m] = W1[H + c*128+p, m] * 16
    w1b = np.ascontiguousarray(
        (W1_w[H:] * FP8_S).reshape(3, 128, H).transpose(1, 0, 2)
    ).reshape(128, E_IN).astype(F8)

    shared = {
        "w1adr": w1adr,
        "w1b": w1b,
        "w2": np.asarray(W2_w, f).astype(BF),
        "w3": (np.asarray(W3_w, f) / SCALE).astype(BF),
        "win": np.asarray(Win_w, f).astype(BF),
        "wout": np.ascontiguousarray(
            np.asarray(Wout_w, f).reshape(4, 128, H).transpose(1, 0, 2)
        ).reshape(128, 4 * H).astype(BF),
        "b1": np.asarray(W1_b, f).reshape(H, 1),
        "b2": np.asarray(W2_b, f).reshape(H, 1),
        "b3x48": np.ascontiguousarray(
            np.broadcast_to(np.asarray(W3_b, f)[None, :] / SCALE, (K, H))
        ).astype(BF),
        "binc": np.ascontiguousarray(np.asarray(Win_b, f).reshape(4, 128).T),
        "bincr": np.ascontiguousarray(np.asarray(Win_b, f).reshape(4, 128)).astype(
            __import__("ml_dtypes").bfloat16
        ),
        "bout": np.asarray(Wout_b, f).reshape(H, 1),
        "s1b": np.ascontiguousarray(
            np.broadcast_to(np.asarray(norm1_s, f)[None, :], (128, H))
        ),
        "o1b": np.ascontiguousarray(
            np.broadcast_to(np.asarray(norm1_o, f)[None, :], (128, H))
        ),
        "ident": np.eye(128, dtype=f),
    }
    mvb = mask_V.reshape(NCORES, NBLK, 128, 1)
    s2mv = np.ascontiguousarray(mvb * np.asarray(norm2_s, f)[None, None, None, :])
    o2mv = np.ascontiguousarray(mvb * np.asarray(norm2_o, f)[None, None, None, :])
    in_maps = []
    for c in range(NCORES):
        m = {
            "xe": xe[c],
            "hvdr": hvdr[c],
            "hvt": hvt[c],
            "s2mv": s2mv[c],
            "o2mv": o2mv[c],
        }
        m.update(shared)
        in_maps.append(m)
    return in_maps


# ---------------- general masked fallback (slow path) ----------------

def _build_nc_masked(xe_bf16=True, reps=1):
    import concourse.bass as bass
    import concourse.mybir as mybir
    from concourse import bacc
    from concourse.bass import ts
    from concourse.tile import TileContext
    from contextlib import ExitStack

    F32 = mybir.dt.float32
    F32R = mybir.dt.float32r
    BF16 = mybir.dt.bfloat16
    XDT = BF16 if xe_bf16 else F32R
    XDDT = BF16 if xe_bf16 else F32
    GELU = mybir.ActivationFunctionType.Gelu
    SQRT = mybir.ActivationFunctionType.Sqrt
    ADD = mybir.AluOpType.add
    AX = mybir.AxisListType.X

    nc = bacc.Bacc()

    xe = nc.dram_tensor("xe", [NBLK, E_IN, TPB], XDDT, kind="ExternalInput")
    hvf = nc.dram_tensor("hvf", [H, NPC], XDDT, kind="ExternalInput")
    hvt = nc.dram_tensor("hvt", [NBLK, 128, H], F32, kind="ExternalInput")
    mkm = nc.dram_tensor("mkm", [NBLK, TPB], XDDT, kind="ExternalInput")
    mvv = nc.dram_tensor("mv", [NBLK, 128, 1], F32, kind="ExternalInput")
    w1a = nc.dram_tensor("w1a", [H, H], XDDT, kind="ExternalInput")
    w1b = nc.dram_tensor("w1b", [128, E_IN], XDDT, kind="ExternalInput")
    w2 = nc.dram_tensor("w2", [H, H], XDDT, kind="ExternalInput")
    w3 = nc.dram_tensor("w3", [H, H], F32, kind="ExternalInput")
    win = nc.dram_tensor("win", [H, 4 * H], XDDT, kind="ExternalInput")
    wout = nc.dram_tensor("wout", [128, 4 * H], XDDT, kind="ExternalInput")
    b1 = nc.dram_tensor("b1", [H, 1], F32, kind="ExternalInput")
    b2 = nc.dram_tensor("b2", [H, 1], F32, kind="ExternalInput")
    b3x48 = nc.dram_tensor("b3x48", [K, H], XDDT, kind="ExternalInput")
    binc = nc.dram_tensor("binc", [128, 4], F32, kind="ExternalInput")
    bincr = nc.dram_tensor("bincr", [4, 128], BF16, kind="ExternalInput")
    bout = nc.dram_tensor("bout", [H, 1], F32, kind="ExternalInput")
    s1b = nc.dram_tensor("s1b", [128, H], F32, kind="ExternalInput")
    o1b = nc.dram_tensor("o1b", [128, H], F32, kind="ExternalInput")
    s2b = nc.dram_tensor("s2b", [128, H], F32, kind="ExternalInput")
    o2b = nc.dram_tensor("o2b", [128, H], F32, kind="ExternalInput")
    ident = nc.dram_tensor("ident", [128, 128], F32, kind="ExternalInput")
    ones1 = nc.dram_tensor("ones1", [1, 128], XDDT, kind="ExternalInput")
    out = nc.dram_tensor("out", [NPC, H], F32, kind="ExternalOutput")

    with TileContext(nc) as tc, ExitStack() as ctx:
        const = ctx.enter_context(tc.tile_pool(name="const", bufs=1))
        xep = [
            ctx.enter_context(tc.tile_pool(name=f"xe{c}", bufs=3 if xe_bf16 else 2))
            for c in range(3)
        ]
        g1p = ctx.enter_context(tc.tile_pool(name="g1", bufs=4))
        g2p = ctx.enter_context(tc.tile_pool(name="g2", bufs=4))
        g2mp = ctx.enter_context(tc.tile_pool(name="g2m", bufs=4))
        rpp = ctx.enter_context(tc.tile_pool(name="rp", bufs=3))
        rbp = ctx.enter_context(tc.tile_pool(name="rb", bufs=2))
        hvrp = ctx.enter_context(tc.tile_pool(name="hvr", bufs=2))
        mkp = ctx.enter_context(tc.tile_pool(name="mk", bufs=2 if xe_bf16 else 1))
        smp = ctx.enter_context(tc.tile_pool(name="sm", bufs=3))
        x1p = ctx.enter_context(tc.tile_pool(name="x1", bufs=5))
        outp = ctx.enter_context(tc.tile_pool(name="outp", bufs=2))
        psA = ctx.enter_context(tc.tile_pool(name="psA", bufs=3, space="PSUM"))
        psB = ctx.enter_context(tc.tile_pool(name="psB", bufs=2, space="PSUM"))
        psC = ctx.enter_context(tc.tile_pool(name="psC", bufs=2, space="PSUM"))
        psT = ctx.enter_context(tc.tile_pool(name="psT", bufs=1, space="PSUM"))

        def cload(name, dram, shape, dt):
            t = const.tile(shape, dt, tag=name)
            src = dram[:]
            if dt == F32R:
                src = src.bitcast(F32R)
            nc.sync.dma_start(out=t, in_=src)
            return t

        w1a_t = cload("w1a", w1a, [H, H], XDT)
        w1b_t = cload("w1b", w1b, [128, E_IN], XDT)
        w2_t = cload("w2", w2, [H, H], XDT)
        w3_t = cload("w3", w3, [H, H], F32)
        win_t = cload("win", win, [H, 4 * H], XDT)
        wout_t = cload("wout", wout, [128, 4 * H], XDT)
        b1_t = cload("b1", b1, [H, 1], F32)
        b2_t = cload("b2", b2, [H, 1], F32)
        b3x48_t = cload("b3x48", b3x48, [K, H], XDT if xe_bf16 else F32)
        binc_t = cload("binc", binc, [128, 4], F32)
        bincr_t = cload("bincr", bincr, [4, 128], BF16)
        bout_t = cload("bout", bout, [H, 1], F32)
        s1b_t = cload("s1b", s1b, [128, H], F32)
        o1b_t = cload("o1b", o1b, [128, H], F32)
        s2b_t = cload("s2b", s2b, [128, H], F32)
        o2b_t = cload("o2b", o2b, [128, H], F32)
        ident_t = cload("ident", ident, [128, 128], F32)
        ones1_t = cload("ones1", ones1, [1, 128], XDT)
        hvf_t = cload("hvf", hvf, [H, NPC], XDT)

        x1F = const.tile([H, NPC], XDT, tag="x1F")
        z_sb = const.tile([128, 4, NPC], XDT, tag="z_sb")
        eps_t = const.tile([128, 1], F32, tag="eps")
        nc.vector.memset(eps_t, EPS)

        for _rep in range(reps):
            x1_tiles = []
            for b in range(NBLK):
                mkr = mkp.tile([1, TPB], XDT, tag="mkr")
                mkr_src = mkm[b : b + 1, :]
                if not xe_bf16:
                    mkr_src = mkr_src.bitcast(F32R)
                nc.sync.dma_start(out=mkr, in_=mkr_src)
                m48 = smp.tile([K, 128], XDT if xe_bf16 else F32, tag="m48")
                nc.sync.dma_start(
                    out=m48,
                    in_=mkm[b : b + 1, :].rearrange("o (k n) -> (o k) n", k=K),
                )
                hvrep = hvrp.tile([H, 512], XDT, tag="hvrep")
                for i in range(4):
                    nc.gpsimd.tensor_copy(
                        out=hvrep[:, ts(i, 128)], in_=hvf_t[:, ts(b, 128)]
                    )
                hvt_b = smp.tile([128, H], F32, tag="hvt")
                nc.sync.dma_start(out=hvt_b, in_=hvt[b])
                rblk = rbp.tile([H, 128], F32, tag="rblk")

                for half in range(2):
                    xet = []
                    for c in range(3):
                        t = xep[c].tile([128, TPB // 2], XDT, tag=f"xe{c}")
                        src_ap = xe[
                            b, ts(c, 128), half * (TPB // 2) : (half + 1) * (TPB // 2)
                        ]
                        if not xe_bf16:
                            src_ap = src_ap.bitcast(F32R)
                        nc.sync.dma_start(out=t, in_=src_ap)
                        xet.append(t)
                    for jj in range(6):
                        j = half * 6 + jj
                        ps1 = psA.tile([H, 512], F32, tag="ps1")
                        nc.tensor.matmul(ps1[:], w1a_t[:], hvrep[:], start=True, stop=False)
                        for c in range(3):
                            nc.tensor.matmul(
                                ps1[:],
                                w1b_t[:, ts(c, 128)],
                                xet[c][:, ts(jj, 512)],
                                start=False,
                                stop=(c == 2),
                            )
                        g1 = g1p.tile([H, 512], XDT, tag="g1")
                        nc.scalar.activation(out=g1[:], in_=ps1[:], func=GELU, bias=b1_t[:])
                        ps2 = psB.tile([H, 512], F32, tag="ps2")
                        nc.tensor.matmul(ps2[:], w2_t[:], g1[:], start=True, stop=True)
                        g2 = g2p.tile([H, 512], F32, tag="g2")
                        nc.scalar.activation(out=g2[:], in_=ps2[:], func=GELU, bias=b2_t[:])
                        psm = psC.tile([128, 512], F32, tag="psm")
                        nc.tensor.matmul(
                            psm[:],
                            ones1_t[:],
                            mkr[:, ts(j, 512)],
                            start=True,
                            stop=True,
                        )
                        g2m = g2mp.tile([H, 512], F32, tag="g2m")
                        nc.vector.tensor_mul(out=g2m[:], in0=g2[:], in1=psm[:])
                        g2mr = g2m[:].rearrange("p (k n) -> p n k", k=4)
                        if j == 0:
                            nc.vector.tensor_reduce(out=rblk[:], in_=g2mr, axis=AX, op=ADD)
                        else:
                            rp = rpp.tile([H, 128], F32, tag="rp")
                            nc.vector.tensor_reduce(out=rp[:], in_=g2mr, axis=AX, op=ADD)
                            nc.gpsimd.tensor_add(out=rblk[:], in0=rblk[:], in1=rp[:])

                psd = psB.tile([H, 128], F32, tag="ps2")
                nc.tensor.matmul(psd[:], w3_t[:], rblk[:], start=True, stop=False)
                nc.tensor.matmul(psd[:], b3x48_t[:], m48[:], start=False, stop=True)
                dh_sb = smp.tile([H, 128], F32, tag="dh_sb")
                nc.vector.tensor_copy(out=dh_sb[:], in_=psd[:])
                psdT = psT.tile([128, H], F32, tag="psT")
                nc.tensor.transpose(psdT[:], dh_sb[:], ident_t[:])
                x1pre = smp.tile([128, H], F32, tag="x1pre")
                nc.vector.tensor_add(out=x1pre[:], in0=hvt_b[:], in1=psdT[:])
                st6 = smp.tile([128, 6], F32, tag="st6")
                nc.vector.bn_stats(out=st6[:], in_=x1pre[:])
                mv2 = smp.tile([128, 2], F32, tag="mv2")
                nc.vector.bn_aggr(out=mv2[:], in_=st6[:])
                sd = smp.tile([128, 1], F32, tag="sd")
                nc.scalar.activation(out=sd[:], in_=mv2[:, 1:2], func=SQRT, bias=eps_t[:])
                rstd = smp.tile([128, 1], F32, tag="rstd")
                nc.vector.reciprocal(out=rstd[:], in_=sd[:])
                xn = (x1p if triv1 else smp).tile([128, H], F32, tag="xn")
                nc.vector.tensor_scalar(
                    out=xn[:],
                    in0=x1pre[:],
                    scalar1=mv2[:, 0:1],
                    scalar2=rstd[:],
                    op0=mybir.AluOpType.subtract,
                    op1=mybir.AluOpType.mult,
                )
                if triv1:
                    x1 = xn
                else:
                    x1 = x1p.tile([128, H], F32, tag="x1")
                    nc.vector.tensor_mul(out=x1[:], in0=xn[:], in1=s1b_t[:])
                    nc.vector.tensor_add(out=x1[:], in0=x1[:], in1=o1b_t[:])
                x1_tiles.append(x1)
                psxT = psT.tile([H, 128], F32, tag="psT")
                nc.tensor.transpose(psxT[:], x1[:], ident_t[:])
                nc.vector.tensor_copy(out=x1F[:, ts(b, 128)], in_=psxT[:])

            for c in range(4):
                psz = psA.tile([128, NPC], F32, tag="ps1")
                nc.tensor.matmul(psz[:], win_t[:, ts(c, 128)], x1F[:], start=True, stop=True)
                nc.scalar.activation(
                    out=z_sb[:, c, :], in_=psz[:], func=GELU, bias=binc_t[:, c : c + 1]
                )
            psd2 = psB.tile([H, NPC], F32, tag="ps2")
            for c in range(4):
                nc.tensor.matmul(
                    psd2[:],
                    wout_t[:, ts(c, 128)],
                    z_sb[:, c, :],
                    start=(c == 0),
                    stop=(c == 3),
                )
            for b in range(NBLK):
                dh2 = smp.tile([H, 128], F32, tag="dh2")
                nc.vector.tensor_scalar_add(
                    out=dh2[:], in0=psd2[:, ts(b, 128)], scalar1=bout_t[:]
                )
                psd2T = psT.tile([128, H], F32, tag="psT")
                nc.tensor.transpose(psd2T[:], dh2[:], ident_t[:])
                x2 = smp.tile([128, H], F32, tag="x2")
                nc.vector.tensor_add(out=x2[:], in0=x1_tiles[b][:], in1=psd2T[:])
                st6b = smp.tile([128, 6], F32, tag="st6")
                nc.vector.bn_stats(out=st6b[:], in_=x2[:])
                mv2b = smp.tile([128, 2], F32, tag="mv2")
                nc.vector.bn_aggr(out=mv2b[:], in_=st6b[:])
                sdb = smp.tile([128, 1], F32, tag="sd")
                nc.scalar.activation(out=sdb[:], in_=mv2b[:, 1:2], func=SQRT, bias=eps_t[:])
                rstdb = smp.tile([128, 1], F32, tag="rstd")
                nc.vector.reciprocal(out=rstdb[:], in_=sdb[:])
                y = outp.tile([128, H], F32, tag="y")
                nc.vector.tensor_scalar(
                    out=y[:],
                    in0=x2[:],
                    scalar1=mv2b[:, 0:1],
                    scalar2=rstdb[:],
                    op0=mybir.AluOpType.subtract,
                    op1=mybir.AluOpType.mult,
                )
                nc.vector.tensor_mul(out=y[:], in0=y[:], in1=s2b_t[:])
                nc.vector.tensor_add(out=y[:], in0=y[:], in1=o2b_t[:])
                mvb = smp.tile([128, 1], F32, tag="mvb")
                nc.sync.dma_start(out=mvb, in_=mvv[b])
                nc.vector.tensor_scalar_mul(out=y[:], in0=y[:], scalar1=mvb[:])
                nc.sync.dma_start(out=out[ts(b, 128), :], in_=y[:])

    nc.finalize()
    return nc


def _build_nc(xe_bf16=True, reps=1, no_mask=True):
    if no_mask:
        return _build_nc2(reps=reps)
    return _build_nc_masked(xe_bf16=xe_bf16, reps=reps)


XE_BF16 = True  # kept for test.py compatibility (masked fallback dtype)


def _prep_inputs_masked(h_V, h_E, mask_V, mask_attend, W1_w, W1_b, W2_w, W2_b,
                        W3_w, W3_b, Win_w, Win_b, Wout_w, Wout_b,
                        norm1_s, norm1_o, norm2_s, norm2_o):
    import ml_dtypes
    f = np.float32
    BF = ml_dtypes.bfloat16
    h_V = np.asarray(h_V, f)
    h_E = np.asarray(h_E, f)
    mask_V = np.asarray(mask_V, f)
    mask_attend = np.asarray(mask_attend, f)

    xe = np.ascontiguousarray(
        h_E.reshape(NCORES, NBLK, 128, K, E_IN).transpose(0, 1, 4, 3, 2)
    ).reshape(NCORES, NBLK, E_IN, TPB).astype(BF)
    hvf = np.ascontiguousarray(
        h_V.reshape(NCORES, NPC, H).transpose(0, 2, 1)
    ).astype(BF)
    hvt = h_V.reshape(NCORES, NBLK, 128, H)
    mkm = np.ascontiguousarray(
        mask_attend.reshape(NCORES, NBLK, 128, K).transpose(0, 1, 3, 2)
    ).reshape(NCORES, NBLK, TPB).astype(BF)
    mv = np.ascontiguousarray(mask_V.reshape(NCORES, NBLK, 128, 1))

    shared = {
        "w1a": np.ascontiguousarray(np.asarray(W1_w, f)[:H]).astype(BF),
        "w1b": np.ascontiguousarray(
            np.asarray(W1_w, f)[H:].reshape(3, 128, H).transpose(1, 0, 2)
        ).reshape(128, E_IN).astype(BF),
        "w2": np.asarray(W2_w, f).astype(BF),
        "w3": np.asarray(W3_w, f) / SCALE,
        "win": np.asarray(Win_w, f).astype(BF),
        "wout": np.ascontiguousarray(
            np.asarray(Wout_w, f).reshape(4, 128, H).transpose(1, 0, 2)
        ).reshape(128, 4 * H).astype(BF),
        "b1": np.asarray(W1_b, f).reshape(H, 1),
        "b2": np.asarray(W2_b, f).reshape(H, 1),
        "b3x48": np.ascontiguousarray(
            np.broadcast_to(np.asarray(W3_b, f)[None, :] / SCALE, (K, H))
        ).astype(BF),
        "binc": np.ascontiguousarray(np.asarray(Win_b, f).reshape(4, 128).T),
        "bout": np.asarray(Wout_b, f).reshape(H, 1),
        "s1b": np.ascontiguousarray(
            np.broadcast_to(np.asarray(norm1_s, f)[None, :], (128, H))
        ),
        "o1b": np.ascontiguousarray(
            np.broadcast_to(np.asarray(norm1_o, f)[None, :], (128, H))
        ),
        "s2b": np.ascontiguousarray(
            np.broadcast_to(np.asarray(norm2_s, f)[None, :], (128, H))
        ),
        "o2b": np.ascontiguousarray(
            np.broadcast_to(np.asarray(norm2_o, f)[None, :], (128, H))
        ),
        "ident": np.eye(128, dtype=f),
        "ones1": np.ones((1, 128), f).astype(BF),
    }
    in_maps = []
    for c in range(NCORES):
        m = {
            "xe": xe[c],
            "hvf": hvf[c],
            "hvt": hvt[c],
            "mkm": mkm[c],
            "mv": mv[c],
        }
        m.update(shared)
        in_maps.append(m)
    return in_maps


def _prep_inputs(h_V, h_E, mask_V, mask_attend, **kw):
    if np.all(np.asarray(mask_attend) == 1.0):
        return _prep_inputs2(h_V, h_E, mask_V, **kw)
    return _prep_inputs_masked(h_V, h_E, mask_V, mask_attend, **kw)


def run(trace=False, **inputs):
    from concourse.bass_utils import run_bass_kernel_spmd

    no_mask = bool(np.all(np.asarray(inputs["mask_attend"]) == 1.0))
    triv1 = bool(
        np.all(np.asarray(inputs["norm1_s"]) == 1.0)
        and np.all(np.asarray(inputs["norm1_o"]) == 0.0)
    )
    triv2 = bool(
        np.all(np.asarray(inputs["norm2_s"]) == 1.0)
        and np.all(np.asarray(inputs["norm2_o"]) == 0.0)
        and np.all(np.asarray(inputs["mask_V"]) == 1.0)
    )
    trivfb = bool(np.all(np.asarray(inputs["Win_b"]) == 0.0))
    key = ("nc", no_mask, triv1, triv2, trivfb)
    if key not in _CACHE:
        if no_mask:
            _CACHE[key] = _build_nc2(triv1=triv1, triv2=triv2, trivfb=trivfb)
        else:
            _CACHE[key] = _build_nc_masked()
    nc = _CACHE[key]
    in_maps = _prep_inputs(**inputs)
    res = run_bass_kernel_spmd(nc, in_maps, core_ids=list(range(NCORES)), trace=trace)
    outp = np.concatenate([r["out"] for r in res.results], axis=0)
    return outp.astype(np.float32), res


def kernel(**inputs):
    outp, _ = run(trace=False, **inputs)
    return outp
